# revision 35
# baseline (speedup 1.0000x reference)
"""Trainium2 Bass kernel for AttentionFusion (B=4, T=4, H=W=32, C=128).

Sharding: 8 cores = batch (4) x query-half (2). Each core computes full
attention for its 2048 query rows of one batch element against all 4096
keys of that element. No cross-core communication.

Schedule (v2): the 64 exp ops on ACT (the ~71us fundamental floor) run
back-to-back and pace the kernel; PE streams S^T = K_tile^T @ Q and
O += V^T_tile @ P matmuls with the PV pairs trailing exp by 4 groups
(the 32-slot P ring decouples them) so PE never waits on a fresh exp.
K/Q/V^T projections are fused into block 0's slots instead of a serial
prologue (head Q copies run on ACT as Identity+bias, in parallel with
the DVE K copies); input DMAs are issued from the sync/scalar sequencers (which
boot before Pool) with per-transfer semaphores; warm-up matmuls keep
the PE clock ramped during the initial DMA wait.  The softmax rowsum
uses pair+quad adds natively on DVE (2-byte perf modes; walrus
rebalances some to Pool), a ones-column matmul per quad accumulating
in PSUM, and reciprocal on DVE -- except the last block, whose recip
runs as exp(-ln(rs/16)) on the then-idle ACT (same act table) to cut
the tail latency.  The output bias is folded into a second
accumulating matmul (bo_row x rowsum) and the final normalize is one
DVE multiply; per-block output DMA is triggered from Pool.

f16 operands throughout; Wo is pre-scaled by 1/16 on the host so the
f16 reciprocal-broadcast (16/rowsum) stays in f16 normal range.
"""
import sys

sys.path.insert(0, "/opt/trn_rl_repo")

import numpy as np

import concourse.bass as bass
import concourse.mybir as mybir
from concourse.bass_utils import run_bass_kernel_spmd

f32 = mybir.dt.float32
f16 = mybir.dt.float16
f32r = mybir.dt.float32r

B, T, C, H, W = 4, 4, 128, 32, 32
N = T * H * W            # 4096 keys per batch element
NLOC = N // 2            # 2048 query rows per core
NB = 4                   # query blocks of 512
MT = 32                  # key tiles of 128
NG = 16                  # groups per block (2 key tiles each)
GG = NB * NG             # 64 global groups
SCALE = float(C) ** -0.5
N_CORES = 8

PE, ACT, DVE, POOL, SYNC = "pe", "act", "dve", "pool", "sync"


def _build(stage="full"):
    nc = bass.Bass("TRN2")

    xs = nc.declare_dram_parameter("xs", [C, NLOC], f16, isOutput=False)
    xt = nc.declare_dram_parameter("xt", [C, N], f16, isOutput=False)
    w3 = nc.declare_dram_parameter("w3", [C, 3 * C], f16, isOutput=False)  # wqT|wkT|woT
    b3 = nc.declare_dram_parameter("b3", [C, 3], f32, isOutput=False)      # bq|bk|bo_eff
    obr = nc.declare_dram_parameter("obr", [1, 2 * C], f16, isOutput=False)  # ones|bo_eff
    wvob = nc.declare_dram_parameter("wvob", [C, C + 1], f16, isOutput=False)  # ones_col|wvT
    out = nc.declare_dram_parameter("out", [C, NLOC], f32, isOutput=True)

    from contextlib import ExitStack
    ctx = ExitStack()
    with ctx:
        def sb(name, shape, dt):
            return ctx.enter_context(nc.sbuf_tensor(name, shape, dt))

        def ps(name, shape, dt):
            return ctx.enter_context(nc.psum_tensor(name, shape, dt))

        s_xs = sb("s_xs", [C, NLOC], f16)
        s_xt = sb("s_xt", [C, N], f16)
        s_w3 = sb("s_w3", [C, 3 * C], f16)
        s_b3 = sb("s_b3", [C, 3], f32)
        s_obr = sb("s_obr", [1, 2 * C], f16)
        s_wvob = sb("s_wvob", [C, C + 1], f16)
        s_K = sb("s_K", [C, N], f16)
        s_Q = sb("s_Q", [C, NLOC], f16)
        s_VT = sb("s_VT", [C, N], f16)
        s_PT = sb("s_PT", [C, MT * 512], f16)    # 32 ring slots of [128,512]
        s_PS = sb("s_PS", [C, 8 * 512], f16)     # pair-sum ring
        s_QS = sb("s_QS", [C, 4 * 512], f16)     # quad-sum ring
        s_O = sb("s_O", [C, 512], f16)
        s_rb = sb("s_rb", [C, 512], f16)
        s_rs32 = sb("s_rs32", [1, 512], f32)
        s_rs16 = sb("s_rs16", [1, 512], f16)
        s_rc = sb("s_rc", [1, 512], f32)
        s_rc16 = sb("s_rc16", [1, 512], f16)
        s_lnt = sb("s_lnt", [1, 512], f32)
        s_warm = sb("s_warm", [C, 512], f16)
        s_Y = sb("s_Y", [C, NLOC], f32)

        st_ps0 = ps("st_ps0", [C, 1024], f32)
        st_ps1 = ps("st_ps1", [C, 1024], f32)
        o_ps = ps("o_ps", [C, 512], f32)
        rs_ps = ps("rs_ps", [C, 512], f32)   # row 0 = rowsum; full tile in prologue
        rb_ps = ps("rb_ps", [C, 512], f32)
        y_ps = ps("y_ps", [C, 512], f32)
        st_ps = [st_ps0, st_ps1]

        sems = {k: ctx.enter_context(nc.semaphore(f"{k}_sem"))
                for k in [PE, ACT, DVE, POOL,
                          "wts", "w3s", "xt0", "xt1", "xt2", "xt3", "xs0", "xs1", "outs"]}
        block = ctx.enter_context(nc.Block())

        # ------------- schedule builder -------------
        lists = {e: [] for e in (PE, ACT, DVE, POOL, SYNC)}
        counts = {k: 0 for k in sems}
        marks = {}

        def add(eng, emit, waits=(), name=None, dma_sem=None):
            if dma_sem is None:
                semk, amt = eng, 1
            else:
                semk, amt = dma_sem, 16
            counts[semk] += amt
            lists[eng].append((tuple(waits), emit, semk, amt))
            if name is not None:
                marks[name] = (semk, counts[semk])

        def w(*names_or_pairs):
            res = []
            for x in names_or_pairs:
                if isinstance(x, tuple):
                    res.append(x)
                else:
                    res.append(marks[x])
            return res

        def pt(k):
            k = k % MT
            return s_PT[:, k * 512:(k + 1) * 512]

        def pt2(s):
            o = 2 * (s % NG)
            return s_PT[:, o * 512:(o + 2) * 512]

        def vt_slot(i):
            t = [rb_ps, y_ps][i % 2]
            c = ((i // 2) % 2) * 128
            return t[:, c:c + 128]

        def mm(*a, **kw):
            return lambda: nc.tensor.matmul(*a, **kw)

        # ------------- input DMA issue (sync + scalar HW DGE) -------------
        add(SYNC, lambda: nc.sync.dma_start(s_xt[:, 0:1024], xt[:, 0:1024]), dma_sem="xt0")
        add(SYNC, lambda: nc.sync.dma_start(s_b3[:], b3[:]), dma_sem="wts")
        add(SYNC, lambda: nc.sync.dma_start(s_w3[:], w3[:]), dma_sem="w3s")
        add(SYNC, lambda: nc.sync.dma_start(s_xs[:, 0:1024], xs[:, 0:1024]), dma_sem="xs0")
        add(SYNC, lambda: nc.sync.dma_start(s_wvob[:], wvob[:]), dma_sem="wts")
        add(SYNC, lambda: nc.sync.dma_start(s_obr[:], obr[:]), dma_sem="wts")
        add(SYNC, lambda: nc.sync.dma_start(s_xt[:, 2048:3072], xt[:, 2048:3072]), dma_sem="xt2")
        add(ACT, lambda: nc.scalar.dma_start(s_xt[:, 1024:2048], xt[:, 1024:2048]), dma_sem="xt1")
        add(ACT, lambda: nc.scalar.dma_start(s_xs[:, 1024:2048], xs[:, 1024:2048]), dma_sem="xs1")
        add(ACT, lambda: nc.scalar.dma_start(s_xt[:, 3072:4096], xt[:, 3072:4096]), dma_sem="xt3")

        # ACT: dummy exp to trigger the act-table load early
        add(ACT, lambda: nc.scalar.activation(
            s_rb[0:1, 0:8], s_warm[0:1, 0:8],
            mybir.ActivationFunctionType.Exp, scale=1.0), name="expdummy")

        # ------------- prologue: K0/Q0/K1/Q1 only; rest fused into block 0 -------------
        xtsem = ["xt0", "xt0", "xt1", "xt1", "xt2", "xt2", "xt3", "xt3"]
        xssem = ["xs0", "xs0", "xs1", "xs1"]

        def lp(fn):
            def g():
                with nc.allow_low_precision(reason="f32r is bit-identical to f32"):
                    return fn()
            return g

        def vtcol(k):
            # ring-mates (4j,4j+2) and (4j+1,4j+3) are copied together
            j, r = k // 4, k % 4
            return 512 * j + {0: 0, 2: 128, 1: 256, 3: 384}[r]

        def emit_vt(i):
            waits = []
            if i == 0:
                waits.append(("wts", 32))  # wvob
            if i >= 4:
                waits.append(marks[f"vtc{(i - 4) | 2}"])
            waits.append((["xt0", "xt1", "xt2", "xt3"][i // 8], 16))
            add(PE, mm(vt_slot(i), s_xt[:, i * 128:(i + 1) * 128],
                       s_wvob[:, 1:C + 1], start=True, stop=True),
                waits=waits, name=f"vtm{i}")
            if i % 4 >= 2:   # copy bank-mates (i-2, i) as one [128,256]
                add(DVE, (lambda i=i: nc.vector.tensor_copy(
                    s_VT[:, vtcol(i - 2):vtcol(i - 2) + 256],
                    [rb_ps, y_ps][i % 2][:, 0:256])),
                    waits=w(f"vtm{i}"), name=f"vtc{i}")

        def emit_proj(u, kind, j, slot_ring, copy_eng=DVE):
            waits = [("w3s", 16)]
            waits.append(((xtsem[j] if kind == "K" else xssem[j]), 16))
            ring_prev = proj_ring_prev.get(u)
            if ring_prev is not None:
                waits.append(marks[f"pc{ring_prev}"])
            lhs = s_w3[:, C:2 * C] if kind == "K" else s_w3[:, 0:C]
            src_ = s_xt if kind == "K" else s_xs
            add(PE, mm(slot_ring, lhs, src_[:, j * 512:(j + 1) * 512],
                       start=True, stop=True), waits=waits, name=f"pm{u}")
            dst = s_K if kind == "K" else s_Q
            bcol = s_b3[:, 1:2] if kind == "K" else s_b3[:, 0:1]
            cw = w(f"pm{u}") + ([("wts", 16)] if u <= 1 else [])  # b3
            if copy_eng == DVE:
                add(DVE, (lambda dst=dst, j=j, sr=slot_ring, bcol=bcol:
                          nc.vector.tensor_scalar_add(
                              dst[:, j * 512:(j + 1) * 512], sr, bcol)),
                    waits=cw, name=f"pc{u}")
            else:
                # head Q copies run on the still-idle ACT in parallel with
                # the DVE K copies (activation Copy applies the bias)
                add(ACT, (lambda dst=dst, j=j, sr=slot_ring, bcol=bcol:
                          nc.scalar.activation(
                              dst[:, j * 512:(j + 1) * 512], sr,
                              mybir.ActivationFunctionType.Identity, bias=bcol)),
                    waits=cw, name=f"pc{u}")
            marks[f"{kind}chunk{j}"] = marks[f"pc{u}"]

        # users 0-3 on the st banks, emitted before the slot loop
        head_users = [("K", 0), ("Q", 0), ("K", 1), ("Q", 1)]
        # leftovers alternate {o_ps, rs_ps}; emitted 2 per slot at slots 0..3
        tail_users = [("K", 2), ("K", 3), ("K", 4), ("K", 5),
                      ("K", 6), ("K", 7), ("Q", 2), ("Q", 3)]
        proj_ring_prev = {6: 4, 7: 5, 8: 6, 9: 7, 10: 8, 11: 9}

        # p-state warm-up: keep the PE clock ramped during the input-DMA
        # wait; results are garbage and overwritten by start=True
        for i in range(24):
            add(PE, mm(st_ps[i % 2][:, (i % 4 // 2) * 512:(i % 4 // 2) * 512 + 512],
                       s_warm[:, 0:C], s_warm[:], start=True, stop=True))
        for u, (kind, j) in enumerate(head_users):
            slot = st_ps[(u // 2) % 2][:, (u % 2) * 512:(u % 2) * 512 + 512]
            emit_proj(u, kind, j, slot, copy_eng=(ACT if kind == "Q" else DVE))
        # marks for the last users of o_ps / rs_ps (checked by pv(0,0)/rsP(0,0))
        LAST_O_USER, LAST_RS_USER = "pc10", "pc11"

        vt_emitted = 0
        # ------------- extra-op schedule tables -------------
        pe_extra = {}
        dve_extra = {}
        pool_extra = {}
        for nb in range(NB):
            base = nb * NG
            nxt = base + NG
            for q, sl in enumerate([5, 9, 11, 13, 15, 16, 17, 18]):
                pe_extra.setdefault(base + sl, []).append(("rsP", nb, q))
            pe_extra.setdefault(nxt + 8, []).append(("rb", nb))
            pe_extra.setdefault(nxt + 9, []).append(("y", nb))
            pe_extra.setdefault(nxt + 9, []).append(("ybias", nb))
            dve_extra.setdefault(nxt + 3, []).append(("ocopy", nb))
            dve_extra.setdefault(nxt + 3, []).append(("rschain", nb))
            dve_extra.setdefault(nxt + 9, []).append(("rbcopy", nb))
            dve_extra.setdefault(nxt + 10, []).append(("sttmul", nb))
            pool_extra.setdefault(nxt + 10, []).append(("outdma", nb))

        def emit_pe_extra(item):
            kind = item[0]
            if kind == "rsP":
                _, nb, q = item
                waits = w(f"quad{nb}_{q}")
                if q == 0:
                    waits += w(f"rscopy{nb - 1}") if nb >= 1 else w(LAST_RS_USER)
                add(PE, mm(rs_ps[0:1, :], s_wvob[:, 0:1],
                           s_QS[:, (q % 4) * 512:(q % 4) * 512 + 512],
                           start=(q == 0), stop=(q == 7)),
                    waits=waits, name=f"rsP{nb}_{q}")
            elif kind == "rb":
                _, nb = item
                if nb < NB - 1:
                    waits = w(f"recip{nb}")
                    if nb == 0:
                        waits += [("wts", 48)]  # obr
                        waits += w(f"vtc{MT - 1}")  # 31 is a pair-copy name
                    else:
                        waits += w(f"sttmul{nb - 1}")
                    add(PE, mm(rb_ps[:], s_obr[0:1, 0:C], s_rc16[:],
                               start=True, stop=True), waits=waits, name=f"rb{nb}")
                else:
                    waits = w(f"recip{nb}") + w(f"sttmul{nb - 1}")
                    add(PE, mm(rb_ps[:], s_obr[0:1, 0:C], s_rc16[:],
                               start=True, stop=True), waits=waits, name=f"rb{nb}")
            elif kind == "y":
                _, nb = item
                add(PE, mm(y_ps[:], s_w3[:, 2 * C:3 * C], s_O[:],
                           start=True, stop=False),
                    waits=w(f"ocopy{nb}"), name=f"y{nb}")
            elif kind == "ybias":
                _, nb = item
                add(PE, mm(y_ps[:], s_obr[0:1, C:2 * C], s_rs16[:],
                           start=False, stop=True),
                    waits=w(f"castrs{nb}"), name=f"ybias{nb}")

        def emit_dve_extra(item):
            kind = item[0]
            if kind == "ocopy":
                _, nb = item
                waits = w(f"pv{nb * NG + NG - 1}")
                if nb >= 1:
                    waits += w(f"y{nb - 1}")
                add(DVE, lambda: nc.vector.tensor_copy(s_O[:], o_ps[:]),
                    waits=waits, name=f"ocopy{nb}")
            elif kind == "rschain":
                _, nb = item
                waits = w(f"rsP{nb}_7")
                if nb >= 1:
                    waits += w(f"ybias{nb - 1}")
                if nb < NB - 1:
                    add(DVE, lambda: nc.vector.tensor_copy(s_rs32[:], rs_ps[0:1, :]),
                        waits=waits, name=f"rscopy{nb}")
                    add(DVE, lambda: nc.vector.tensor_scalar_mul(s_rs16[:], s_rs32[:], 0.0625),
                        name=f"castrs{nb}")
                    waits = w(f"rb{nb - 1}") if nb >= 1 else []
                    add(DVE, lp(lambda: nc.vector.reciprocal(s_rc16[:], s_rs16[:])),
                        waits=waits, name=f"recip{nb}")
                else:
                    # tail block: recip via exp(-ln(rs)) on the now-idle ACT,
                    # reading the rowsum straight from PSUM
                    add(DVE, (lambda: nc.vector.tensor_scalar_mul(
                        s_rs16[:], rs_ps[0:1, :], 0.0625)),
                        waits=waits, name=f"castrs{nb}")
                    add(ACT, (lambda: nc.scalar.activation(
                        s_lnt[:], rs_ps[0:1, :],
                        mybir.ActivationFunctionType.Ln, scale=0.0625)),
                        waits=w(f"rsP{nb}_7"), name=f"lnrs{nb}")
                    add(ACT, (lambda: nc.scalar.activation(
                        s_rc16[:], s_lnt[:],
                        mybir.ActivationFunctionType.Exp, scale=-1.0)),
                        name=f"recip{nb}")
            elif kind == "rbcopy":
                _, nb = item
                if nb < NB - 1:
                    add(DVE, lambda: nc.vector.tensor_copy(s_rb[:], rb_ps[:]),
                        waits=w(f"rb{nb}"), name=f"rbcopy{nb}")
                else:
                    add(DVE, lambda: nc.vector.tensor_copy(s_rb[:], rb_ps[:]),
                        waits=w(f"rb{nb}"), name=f"rbcopy{nb}")
            elif kind == "sttmul":
                _, nb = item
                if nb < NB - 1:
                    add(DVE, (lambda nb=nb: nc.vector.tensor_mul(
                        s_Y[:, nb * 512:(nb + 1) * 512], y_ps[:], s_rb[:])),
                        waits=w(f"ybias{nb}"), name=f"sttmul{nb}")
                else:
                    add(DVE, (lambda nb=nb: nc.vector.tensor_mul(
                        s_Y[:, nb * 512:nb * 512 + 256], y_ps[:, 0:256], s_rb[:, 0:256])),
                        waits=w(f"ybias{nb}"), name=f"sttmulh0{nb}")
                    add(DVE, (lambda nb=nb: nc.vector.tensor_mul(
                        s_Y[:, nb * 512 + 256:(nb + 1) * 512], y_ps[:, 256:512],
                        s_rb[:, 256:512])), name=f"sttmul{nb}")

        def emit_pool_extra(item):
            kind = item[0]
            if kind == "outdma":
                _, nb = item
                if nb < NB - 1:
                    add(POOL, (lambda nb=nb: nc.gpsimd.dma_start(
                        out[:, nb * 512:(nb + 1) * 512],
                        s_Y[:, nb * 512:(nb + 1) * 512])),
                        waits=w(f"sttmul{nb}"), dma_sem="outs")
                else:
                    add(POOL, (lambda nb=nb: nc.gpsimd.dma_start(
                        out[:, nb * 512:nb * 512 + 256],
                        s_Y[:, nb * 512:nb * 512 + 256])),
                        waits=w(f"sttmulh0{nb}"), dma_sem="outs")
                    add(POOL, (lambda nb=nb: nc.gpsimd.dma_start(
                        out[:, nb * 512 + 256:(nb + 1) * 512],
                        s_Y[:, nb * 512 + 256:(nb + 1) * 512])),
                        waits=w(f"sttmul{nb}"), dma_sem="outs")

        # ------------- main slot loop -------------
        for s in range(GG + 11):
            nb, g = s // NG, s % NG
            # --- PE: ST pair ---
            if s < GG:
                waits = []
                if g == 0:
                    waits += w(f"Qchunk{nb}")
                if s < NG and g % 2 == 0:
                    waits += w(f"Kchunk{g // 2}")
                # st banks held the head proj users: wait for their copies
                if s == 0:
                    waits += w("pc0", "pc1")
                elif s == 1:
                    waits += w("pc2", "pc3")
                if s >= 2:
                    waits.append((ACT, marks[f"exp{s - 2}"][1]))
                t0 = 2 * g
                add(PE, mm(st_ps[s % 2][:, 0:512], s_K[:, t0 * 128:(t0 + 1) * 128],
                           s_Q[:, nb * 512:(nb + 1) * 512], start=True, stop=True),
                    waits=waits)
                add(PE, mm(st_ps[s % 2][:, 512:1024],
                           s_K[:, (t0 + 1) * 128:(t0 + 2) * 128],
                           s_Q[:, nb * 512:(nb + 1) * 512], start=True, stop=True),
                    name=f"st{s}")
            # --- PE: leftover projections (2 per slot at slots 0..3) ---
            if s < 4:
                for t in range(2):
                    u = 4 + 2 * s + t
                    kind, j = tail_users[2 * s + t]
                    emit_proj(u, kind, j, [o_ps, rs_ps][u % 2][:])
            # --- PE: VT spread (2 per slot) ---
            if s < NG and vt_emitted < MT:
                for _ in range(2):
                    if vt_emitted < MT:
                        emit_vt(vt_emitted)
                        vt_emitted += 1
            # --- PE: trailing PV pair for group s-4 ---
            sp = s - 4
            if 0 <= sp < GG:
                nbp, gp = sp // NG, sp % NG
                waits = w(f"exp{sp}")
                if nbp == 0:
                    waits += w(f"vtc{(2 * gp + 1) | 2}")
                k0 = 2 * gp
                if gp == 0:
                    waits += w(f"ocopy{nbp - 1}") if nbp >= 1 else w(LAST_O_USER)
                add(PE, mm(o_ps[:], s_VT[:, vtcol(k0):vtcol(k0) + 128], pt(k0),
                           start=(gp == 0), stop=False), waits=waits)
                add(PE, mm(o_ps[:], s_VT[:, vtcol(k0 + 1):vtcol(k0 + 1) + 128], pt(k0 + 1),
                           start=False, stop=(gp == NG - 1)), name=f"pv{sp}")
            # --- PE extras ---
            for item in pe_extra.get(s, []):
                emit_pe_extra(item)
            # --- ACT: exp ---
            if s < GG:
                waits = w(f"st{s}")
                if s >= NG:
                    waits += w(f"pair{s - NG}")
                add(ACT, (lambda s=s: nc.scalar.activation(
                    pt2(s), st_ps[s % 2][:],
                    mybir.ActivationFunctionType.Exp, scale=SCALE)),
                    waits=waits, name=f"exp{s}")
            # --- pair add for group s-1 (Pool; last 2 of each block on DVE) ---
            sq = s - 1
            if 0 <= sq < GG:
                pq = sq % NG
                waits = w(f"exp{sq}")
                add(DVE, (lambda pq=pq: nc.vector.tensor_add(
                    s_PS[:, (pq % 8) * 512:(pq % 8) * 512 + 512],
                    pt(2 * pq), pt(2 * pq + 1))),
                    waits=waits, name=f"pair{sq}")
            # --- POOL: quad add for groups (s-2, s-1 pairs) ---
            sqd = s - 2
            if 0 <= sqd < GG and sqd % 2 == 0:
                qnb, qq = sqd // NG, (sqd % NG) // 2
                waits = []
                if qq >= 4:
                    waits += w(f"rsP{qnb}_{qq - 4}")
                elif qnb >= 1:
                    waits += w(f"rsP{qnb - 1}_{qq + 4}")
                add(DVE, (lambda qq=qq: nc.vector.tensor_add(
                    s_QS[:, (qq % 4) * 512:(qq % 4) * 512 + 512],
                    s_PS[:, ((2 * qq) % 8) * 512:((2 * qq) % 8) * 512 + 512],
                    s_PS[:, ((2 * qq + 1) % 8) * 512:((2 * qq + 1) % 8) * 512 + 512])),
                    waits=waits, name=f"quad{qnb}_{qq}")
            # --- DVE extras ---
            for item in dve_extra.get(s, []):
                emit_dve_extra(item)
            # --- POOL extras ---
            for item in pool_extra.get(s, []):
                emit_pool_extra(item)

        # ------------- emit per engine -------------
        def emit_list(handle, eng):
            for waits, fn, semk, amt in lists[eng]:
                for (wsem, wval) in waits:
                    handle.wait_ge(sems[wsem], wval)
                fn().then_inc(sems[semk], amt)

        @block.sync
        def _(sync):
            emit_list(sync, SYNC)

        @block.tensor
        def _(tensor):
            emit_list(tensor, PE)

        @block.scalar
        def _(scalar):
            emit_list(scalar, ACT)

        @block.vector
        def _(vector):
            emit_list(vector, DVE)

        @block.gpsimd
        def _(gpsimd):
            emit_list(gpsimd, POOL)

    return nc


def _make_in_maps(spatial_features, temporal_features, Wq, bq, Wk, bk, Wv, bv, Wo, bo):
    f = np.float32
    bf = np.float16
    w3 = np.ascontiguousarray(np.concatenate([Wq.T, Wk.T, Wo.T / 16.0], axis=1)).astype(bf)
    bo_eff = (Wo @ bv + bo).astype(f)
    b3 = np.ascontiguousarray(np.stack([bq, bk, bo_eff], axis=1)).astype(f)  # [C, 3]
    obr = np.ascontiguousarray(
        np.concatenate([np.ones((C,), f), bo_eff])[None, :]).astype(bf)      # [1, 2C]
    wvob = np.ascontiguousarray(
        np.concatenate([np.ones((C, 1), f), Wv.T], axis=1)).astype(bf)

    in_maps = []
    for core in range(N_CORES):
        b, half = core // 2, core % 2
        xs_ = np.ascontiguousarray(
            spatial_features[b, 2 * half:2 * half + 2]      # [2, C, H, W]
            .transpose(1, 0, 2, 3).reshape(C, NLOC)).astype(bf)
        xt_ = np.ascontiguousarray(temporal_features[b].reshape(C, N)).astype(bf)
        in_maps.append({
            "xs": xs_,
            "xt": xt_,
            "w3": w3,
            "b3": b3,
            "obr": obr,
            "wvob": wvob,
        })
    return in_maps


_CACHED = {}


def _run(in_maps, trace=False):
    import os
    stage = os.environ.get("KSTAGE", "full")
    if _CACHED.get("stage") != stage:
        _CACHED["nc"] = _build(stage)
        _CACHED["stage"] = stage
    return run_bass_kernel_spmd(_CACHED["nc"], in_maps, list(range(N_CORES)), trace=trace)


def kernel(spatial_features, temporal_features, Wq, bq, Wk, bk, Wv, bv, Wo, bo):
    args = [np.asarray(a) for a in (spatial_features, temporal_features,
                                    Wq, bq, Wk, bk, Wv, bv, Wo, bo)]
    in_maps = _make_in_maps(*args)
    res = _run(in_maps)
    out = np.empty((B, C, T, H, W), np.float32)
    for core in range(N_CORES):
        b, half = core // 2, core % 2
        y = res.results[core]["out"]                        # [C, NLOC]
        out[b, :, 2 * half:2 * half + 2] = np.asarray(y).reshape(C, 2, H, W)
    return out
